# revision 26
# baseline (speedup 1.0000x reference)
"""BertSelfAttention on 8 Trainium2 NeuronCores.

Problem: B=2, S=2048, H=1024, 16 heads x 64. Sharding: batch x head-group
(2 batches x 4 head-groups of 4 heads = 8 cores). Each core computes
q/k/v projections for its 4 heads and full attention over them.

The kernel is ACT(exp)-bound: 128 EXP activations of [128,1024] ~= 152us.
Design keeps ScalarE saturated from ~23us on and everything else in its
shadow:

  - q is processed in 512-wide blocks: 8 stages (hp x qc) x 16 key
    chunks = 128 blocks, one FD-1024 exp per block (both heads of the
    pair side by side in one PSUM score tile).
  - PSUM (8 banks): score pair-tile [128,1024] double-buffered (4) +
    PV accumulators 2x[65,512] (2) + chain/transpose scratch 2x[128,512]
    (2). Double-buffered scores mean the next block's score matmuls
    never wait on the current exp -> no ACT stagger, and the two heads'
    K=64 score matmuls run row-tiled concurrently.
  - probs are parked in a deep SBUF fp16 ring; PV matmuls, output tails
    (PSUM -> SBUF -> PE transpose -> 1/denom scale -> HBM) and the
    projection chains (k/q/v built from x on the fly) are scheduled by a
    small deadline/pressure scheduler into the PE slack of each block.
  - the 65th column of each head's V block is 1.0 so PV also accumulates
    the softmax denominator (no max subtraction: scores ~N(0,1)).
  - xT streams in quarters so the first projection chains start ~11us in.

Per-core device kernel (SPMD; matmul operands fp16, accumulation fp32):
  inputs (host-prepared):
    xT    [1024, 2048]  x[b].T, fp16
    wqT/wkT/wvT [1024, 256]  W.T columns for this head group, fp16
    bqk   [128, 4]      q/k biases per o-chunk (per-partition layout)
    bvb   [128, 260]    v bias + ones column, broadcast across partitions
    mb    [128, 16]     additive mask bias per key position ((1-m)*-1e30)
  output:
    out   [2048, 256]   attention output, natural [s, head-local o]
"""

import sys

sys.path.insert(0, "/opt/trn_rl_repo")

import numpy as np

import concourse.bass as bass
import concourse.tile as tile
from concourse.masks import make_identity
from concourse import bacc, mybir
from concourse.bass_utils import run_bass_kernel_spmd

F32 = mybir.dt.float32
F16 = mybir.dt.float16
EXP = mybir.ActivationFunctionType.Exp

B, S, H = 2, 2048, 1024
NH, HD = 16, 64
G = 4                 # head-groups (cores per batch)
NHL = NH // G         # heads per core
O = NHL * HD          # 256 output features per core
IC = H // 128         # 8 contraction chunks
KC = S // 128         # 16 key chunks
QB = 512              # q block width
NQB = S // QB         # 4 q blocks per head-pair
NJ = QB // 128        # 128-row groups per q block
NEG = -1.0e30
PBB = 16              # pb ring depth (fp16 prob tiles, 2KB/partition each)
STAGES = [(hp, qc) for hp in range(2) for qc in range(NQB)]
NBLK = len(STAGES) * KC


def build_nc():
    nc = bacc.Bacc(None, target_bir_lowering=False)
    xT = nc.declare_dram_parameter("xT", [H, S], F16, isOutput=False)
    wqT = nc.declare_dram_parameter("wqT", [H, O], F16, isOutput=False)
    wkT = nc.declare_dram_parameter("wkT", [H, O], F16, isOutput=False)
    wvT = nc.declare_dram_parameter("wvT", [H, O], F16, isOutput=False)
    bqk = nc.declare_dram_parameter("bqk", [128, 4], F32, isOutput=False)
    bvb = nc.declare_dram_parameter("bvb", [128, NHL * (HD + 1)], F16,
                                    isOutput=False)
    mb = nc.declare_dram_parameter("mb", [128, KC], F32, isOutput=False)
    out = nc.declare_dram_parameter("out", [S, O], F32, isOutput=True)

    with tile.TileContext(nc) as tc:
        with tc.tile_pool(name="consts", bufs=1) as consts, \
             tc.tile_pool(name="persist", bufs=1) as persist, \
             tc.tile_pool(name="xtw", bufs=1) as xtw, \
             tc.tile_pool(name="pbp", bufs=1) as pbp, \
             tc.tile_pool(name="tailp", bufs=1) as tailp, \
             tc.tile_pool(name="scps", bufs=1, space="PSUM") as scps, \
             tc.tile_pool(name="pvps", bufs=1, space="PSUM") as pvps, \
             tc.tile_pool(name="chps", bufs=1, space="PSUM") as chps:
            ident = consts.tile([128, 128], F32, tag="ident")
            make_identity(nc, ident)
            mb_sb = consts.tile([128, KC], F32, tag="mb")
            bqk_sb = consts.tile([128, 4], F32, tag="bqk")
            bvb_sb = consts.tile([128, NHL * (HD + 1)], F16, tag="bvb")

            qT = [persist.tile([128, S], F16, tag=f"qT{i}", name=f"qT{i}")
                  for i in range(2)]
            kT = [persist.tile([128, S], F16, tag=f"kT{i}", name=f"kT{i}")
                  for i in range(2)]
            vS = [persist.tile([128, NHL * (HD + 1)], F16, tag=f"v{i}",
                               name=f"v{i}") for i in range(KC)]

            xt = [xtw.tile([128, S], F16, tag=f"xt{i}", name=f"xt{i}")
                  for i in range(IC)]
            wq = [xtw.tile([128, O], F16, tag=f"wq{i}", name=f"wq{i}")
                  for i in range(IC)]
            wk = [xtw.tile([128, O], F16, tag=f"wk{i}", name=f"wk{i}")
                  for i in range(IC)]
            wv = [xtw.tile([128, O], F16, tag=f"wv{i}", name=f"wv{i}")
                  for i in range(IC)]

            # consts first (tiny, everything retires against them), then
            # x in quarters; first chains need only cols 0-512
            nc.sync.dma_start(out=mb_sb, in_=mb[:, :])
            nc.sync.dma_start(out=bqk_sb, in_=bqk[:, :])
            nc.sync.dma_start(out=bvb_sb, in_=bvb[:, :])
            dummy = consts.tile([128, 1], F32, tag="dummy")
            nc.vector.memset(dummy, 0.0)
            nc.scalar.activation(dummy, dummy, EXP)
            for i in range(IC):
                nc.sync.dma_start(out=xt[i][:, 0:512],
                                  in_=xT[i * 128:(i + 1) * 128, 0:512])
                nc.sync.dma_start(out=wk[i], in_=wkT[i * 128:(i + 1) * 128, :])
                nc.sync.dma_start(out=wq[i], in_=wqT[i * 128:(i + 1) * 128, :])
            for i in range(IC):
                nc.sync.dma_start(out=xt[i][:, 512:1024],
                                  in_=xT[i * 128:(i + 1) * 128, 512:1024])
            for i in range(IC):
                nc.sync.dma_start(out=wv[i], in_=wvT[i * 128:(i + 1) * 128, :])
            for i in range(IC):
                nc.sync.dma_start(out=xt[i][:, 1024:1536],
                                  in_=xT[i * 128:(i + 1) * 128, 1024:1536])
            for i in range(IC):
                nc.sync.dma_start(out=xt[i][:, 1536:2048],
                                  in_=xT[i * 128:(i + 1) * 128, 1536:2048])

            # ---- projection chains (use the chain PSUM slots) ----
            slot = [0]

            def next_slot():
                slot[0] ^= 1
                return slot[0]

            def qk_half(wt, ot, sc, dest, bcol, h, state):
                # half a projection chain (4 of 8 contraction matmuls) so
                # the scheduler can slip score matmuls between the halves
                if h == 0:
                    state["ps"] = chps.tile(
                        [128, 512], F32, tag=f"ch{next_slot()}",
                        name=f"qkc{bcol}_{sc}")
                ps = state["ps"]
                for i in range(h * 4, h * 4 + 4):
                    nc.tensor.matmul(
                        ps,
                        lhsT=wt[i][:, ot * 128:(ot + 1) * 128],
                        rhs=xt[i][:, sc * 512:(sc + 1) * 512],
                        start=(i == 0), stop=(i == IC - 1))
                if h == 1:
                    nc.vector.tensor_scalar_add(
                        dest[:, sc * 512:(sc + 1) * 512], ps,
                        bqk_sb[:, bcol:bcol + 1])

            def qk_chain(wt, ot, sc, dest, bcol):
                state = {}
                qk_half(wt, ot, sc, dest, bcol, 0, state)
                qk_half(wt, ot, sc, dest, bcol, 1, state)

            bvview = bvb_sb.rearrange("p (h d) -> p h d", h=NHL)

            def v_chain(sc):
                ps = chps.tile([128, O], F32, tag=f"ch{next_slot()}",
                               name=f"vch{sc}")
                for i in range(IC):
                    nc.tensor.matmul(
                        ps,
                        lhsT=xt[i][:, sc * 128:(sc + 1) * 128],
                        rhs=wv[i],
                        start=(i == 0), stop=(i == IC - 1))
                vview = vS[sc].rearrange("p (h d) -> p h d", h=NHL)
                nc.vector.tensor_add(
                    vview[:, :, 0:HD],
                    ps.rearrange("p (h d) -> p h d", h=NHL),
                    bvview[:, :, 0:HD])
                nc.vector.tensor_copy(
                    vview[:, :, HD:HD + 1], bvview[:, :, HD:HD + 1])

            # ---- attention building blocks ----
            pb_ref = {}

            def score_block(si, kc):
                hp, qc = STAGES[si]
                scp = scps.tile([128, 2 * QB], F32, tag="scp", bufs=2,
                                name=f"scp{si}_{kc}")
                for e in range(2):
                    lo = e * 64
                    nc.tensor.matmul(
                        scp[:, e * QB:(e + 1) * QB],
                        lhsT=kT[hp][lo:lo + 64, kc * 128:(kc + 1) * 128],
                        rhs=qT[hp][lo:lo + 64, qc * QB:(qc + 1) * QB],
                        start=True, stop=True)
                pb = pbp.tile([128, 2 * QB], F16, tag="pb", bufs=PBB,
                              name=f"pb{si}_{kc}")
                nc.scalar.activation(pb, scp, EXP,
                                     bias=mb_sb[:, kc:kc + 1], scale=0.125)
                pb_ref[(si, kc)] = pb

            pv_t = {}

            def pv_pair(si, kc):
                hp, qc = STAGES[si]
                if kc == 0:
                    pv_t[si] = [
                        pvps.tile([HD + 1, QB], F32, tag=f"pv{e}",
                                  name=f"pv{si}_{e}") for e in range(2)]
                pb = pb_ref.pop((si, kc))
                for e in range(2):
                    hh = 2 * hp + e
                    nc.tensor.matmul(
                        pv_t[si][e],
                        lhsT=vS[kc][:, hh * 65:hh * 65 + 65],
                        rhs=pb[:, e * QB:(e + 1) * QB],
                        start=(kc == 0), stop=(kc == KC - 1))

            def tail_e(si, e):
                hp, qc = STAGES[si]
                hh = 2 * hp + e
                t = pv_t[si][e]
                ovt = tailp.tile([HD + 1, QB], F32, tag=f"ovt{e}",
                                 name=f"ovt{si}{e}")
                nc.vector.tensor_copy(ovt, t)
                tr = chps.tile([128, NJ, 128], F32, tag=f"ch{next_slot()}",
                               name=f"tr{si}{e}")
                for jb in range(NJ):
                    nc.tensor.transpose(
                        tr[:, jb, 0:HD + 1],
                        ovt[:, jb * 128:(jb + 1) * 128],
                        ident[0:HD + 1, 0:HD + 1])
                rc = tailp.tile([128, NJ], F32, tag=f"rc{e}",
                                name=f"rc{si}{e}")
                nc.vector.reciprocal(rc, tr[:, :, HD])
                osb = tailp.tile([128, NJ, HD], F32, tag=f"osb{e}",
                                 name=f"osb{si}{e}")
                for jb in range(NJ):
                    nc.vector.tensor_scalar_mul(
                        osb[:, jb, :], tr[:, jb, 0:HD], rc[:, jb:jb + 1])
                dst = out[qc * QB:(qc + 1) * QB, hh * HD:(hh + 1) * HD]
                dst = dst.rearrange("(j p) d -> p j d", p=128)
                nc.sync.dma_start(out=dst, in_=osb)

            # ---- work queues ----
            # pv FIFO: strictly ordered (accumulators + tails share slots)
            pvq = []
            for si in range(len(STAGES)):
                for kc in range(KC):
                    pvq.append(("pv", si, kc, 0.48))
                pvq.append(("tail", si, 0, 1.10))
                pvq.append(("tail", si, 1, 1.10))
            pvq = pvq[::-1]  # pop() from the end

            # deadline-sorted qk chains (deadline = global block index)
            chq = [
                (3, (wk, 0, 1, kT[0], 2)),
                (7, (wk, 0, 2, kT[0], 2)),
                (11, (wk, 0, 3, kT[0], 2)),
                (14, (wq, 0, 1, qT[0], 0)),
                (30, (wq, 0, 2, qT[0], 0)),
                (46, (wq, 0, 3, qT[0], 0)),
                (56, (wk, 1, 0, kT[1], 3)),
                (60, (wk, 1, 1, kT[1], 3)),
                (62, (wq, 1, 0, qT[1], 1)),
                (69, (wk, 1, 2, kT[1], 3)),
                (73, (wk, 1, 3, kT[1], 3)),
                (78, (wq, 1, 1, qT[1], 1)),
                (94, (wq, 1, 2, qT[1], 1)),
                (110, (wq, 1, 3, qT[1], 1)),
            ]
            chq = chq[::-1]
            vq = list(range(KC))[::-1]   # v chains, natural order
            v_done = set()

            def emit_v(kc):
                v_chain(kc)
                v_done.add(kc)

            max_lag = [0]

            def pv_ready(b):
                if not pvq:
                    return False
                kind, si, x, _ = pvq[-1]
                if kind == "tail":
                    return True
                if si * KC + x + 2 > b:
                    return False      # pb not produced/expd yet
                if si == 0 and x not in v_done:
                    return False      # v chain not yet emitted
                return True

            def pop_pv(b):
                kind, si, x, cost = pvq.pop()
                if kind == "pv":
                    max_lag[0] = max(max_lag[0], b - (si * KC + x))
                    pv_pair(si, x)
                else:
                    tail_e(si, x)
                return cost

            # ---- prefix: first chains so scores(0,0,0) can run ----
            qk_chain(wk, 0, 0, kT[0], 2)
            qk_chain(wq, 0, 0, qT[0], 0)

            # ---- main loop: 128 blocks ----
            pend = []   # second halves of split projection chains
            for b in range(NBLK):
                si, kc = b // KC, b % KC
                score_block(si, kc)
                # blocks 0-1: scores only, so the first exps aren't queued
                # behind DMA-paced chain matmuls
                budget = 0.65 if b >= 2 else 0.0
                # pv backlog pressure: must drain ~0.60us/blk on average
                rem = NBLK - b
                left = sum(c for (_k, _s, _x, c) in pvq)
                if left > rem * 0.62:
                    budget = 0.95 if b >= 2 else 0.0
                while budget > 0:
                    if pend:
                        args, h, state = pend.pop()
                        qk_half(*args, h, state)
                        budget -= 0.9
                    elif chq and chq[-1][0] <= b + 1:
                        _d, args = chq.pop()
                        st = {}
                        qk_half(*args, 0, st)
                        pend.append((args, 1, st))
                        budget -= 0.9
                    elif pvq and pvq[-1][0] == "pv" and pvq[-1][1] == 0 \
                            and pvq[-1][2] not in v_done and vq \
                            and (b >= si * KC + pvq[-1][2]):
                        emit_v(vq.pop())
                        budget -= 1.1
                    elif pv_ready(b) and (left > rem * 0.56 or not vq):
                        budget -= pop_pv(b)
                        left = sum(c for (_k, _s, _x, c) in pvq)
                    elif vq:
                        emit_v(vq.pop())
                        budget -= 1.1
                    elif chq:
                        _d, args = chq.pop()
                        st = {}
                        qk_half(*args, 0, st)
                        pend.append((args, 1, st))
                        budget -= 0.9
                    elif pv_ready(b):
                        budget -= pop_pv(b)
                    else:
                        break

            # ---- drain whatever is left ----
            while pend:
                args, h, state = pend.pop()
                qk_half(*args, h, state)
            while vq:
                emit_v(vq.pop())
            while chq:
                _d, args = chq.pop()
                qk_chain(*args)
            while pvq:
                pop_pv(NBLK)

            assert not pend, "dangling chain half"
            assert not pb_ref, f"unconsumed pb tiles: {list(pb_ref)[:4]}"
            assert max_lag[0] < PBB - 4, f"pb ring too shallow: {max_lag[0]}"
    nc.finalize()
    return nc


_NC_CACHE = None


def _get_nc():
    global _NC_CACHE
    if _NC_CACHE is None:
        _NC_CACHE = build_nc()
    return _NC_CACHE


def make_in_maps(inputs, attention_mask, Wq, bq, Wk, bk, Wv, bv):
    x = np.asarray(inputs, dtype=np.float32)
    mask = np.asarray(attention_mask)
    Wq = np.asarray(Wq, dtype=np.float32)
    Wk = np.asarray(Wk, dtype=np.float32)
    Wv = np.asarray(Wv, dtype=np.float32)
    bq = np.asarray(bq, dtype=np.float32)
    bk = np.asarray(bk, dtype=np.float32)
    bv = np.asarray(bv, dtype=np.float32)

    xTb = [np.ascontiguousarray(x[b].T).astype(np.float16) for b in range(B)]
    mbb = [np.ascontiguousarray(
        ((1.0 - mask[b].astype(np.float32)) * NEG).reshape(KC, 128).T)
        for b in range(B)]
    in_maps = []
    for c in range(8):
        b, g = c // G, c % G
        cols = slice(g * O, (g + 1) * O)
        bqs, bks = bq[cols], bk[cols]
        bvc = np.concatenate(
            [np.concatenate([bv[cols][h * 64:(h + 1) * 64], [1.0]])
             for h in range(NHL)]).astype(np.float32)
        bvbc = np.ascontiguousarray(np.broadcast_to(bvc[None, :], (128, len(bvc))))
        in_maps.append({
            "xT": xTb[b],
            "wqT": np.ascontiguousarray(Wq.T[:, cols]).astype(np.float16),
            "wkT": np.ascontiguousarray(Wk.T[:, cols]).astype(np.float16),
            "wvT": np.ascontiguousarray(Wv.T[:, cols]).astype(np.float16),
            "bqk": np.ascontiguousarray(
                np.stack([bqs[:128], bqs[128:], bks[:128], bks[128:]], axis=1)),
            "bvb": bvbc.astype(np.float16),
            "mb": mbb[b],
        })
    return in_maps


def assemble(results):
    outs = [results[c]["out"] for c in range(8)]
    full = np.stack(
        [np.concatenate(outs[b * G:(b + 1) * G], axis=1) for b in range(B)])
    return np.ascontiguousarray(full.astype(np.float32))


def kernel(**inputs) -> np.ndarray:
    nc = _get_nc()
    in_maps = make_in_maps(**inputs)
    res = run_bass_kernel_spmd(nc, in_maps, core_ids=list(range(8)))
    return assemble(res.results)
